# revision 63
# baseline (speedup 1.0000x reference)
"""Multi-head attention (b=16, n=512, d=768, h=12) on 8 trn2 NeuronCores.

Strategy: pure data-parallel over batch (2 batches per core), no collectives.

QKV projection runs in fp8e4m3 DoubleRow mode (2x contraction per partition,
0.5 PE cycles per output row): host splits x and 64*Wqkv into hi+lo e4m3
planes; the projection computes xh@wh + xh@wl + xl@wh (the dropped lo@lo
term is ~0.06% relative), a 1.33x PE saving over bf16 at bf16-level accuracy.
The 64x weight pre-scale keeps the lo planes out of e4m3's subnormal range;
the 1/64 unscale rides the existing PSUM->SBUF copies for free.

Per-core dataflow (P = 128 partitions):
  qkT[m]  = (Wqkv hi/lo)^T @ (x hi/lo)  DoubleRow -> [feat, tok] bf16 (+bias)
  v_aug   = x @ Wv stored per head as [v_h | ones64]  (natural [tok, feat])
  scoresT = k_h @ q_h^T   (bf16, 2 heads -> one 2-bank PSUM tile)
  attnT   = exp(0.125 * scoresT)  (one [128,1024] activation per 2 banks)
  ctx_h   = v_aug_h^T @ attnT: rows 0-63 = ctxT, rows 64-127 = colsum
  bc      = 1/colsum; ctxT = ctx * bc (fused on the PSUM->SBUF copy)
  out     = ctxT^T @ Wo + bo  (bf16, natural [tok, feat], DMA out)
"""

import numpy as np
import ml_dtypes

import concourse.bass as bass
import concourse.mybir as mybir
import concourse.tile as tile
from concourse import bacc
from concourse.bass_utils import run_bass_kernel_spmd

# Problem constants (hardcoded per contest contract).
B = 16          # global batch
N = 512         # sequence length
D = 768         # embed dim
H = 12          # heads
DH = 64         # head dim
NCORES = 8
BPC = B // NCORES          # batches per core = 2
TOK = BPC * N              # tokens per core = 1024
P = 128
DC = 3                     # fp8 DoubleRow double-chunks over D (3 x 256)
TT = TOK // P              # 8 token tiles
HPAIRS = H // 2            # 6 head pairs
WS = 64.0                  # weight pre-scale (keeps fp8 lo plane normal)

F32 = mybir.dt.float32
BF16 = mybir.dt.bfloat16
F8 = mybir.dt.float8e4
BF16_NP = ml_dtypes.bfloat16
F8_NP = ml_dtypes.float8_e4m3
DR = mybir.MatmulPerfMode.DoubleRow

# Module-level knobs (test.py pokes these; harness uses defaults).
TRACE = False
LAST_EXEC_NS = None
LAST_RESULTS = None

_CACHED_NC = None


def _build_nc():
    # Bacc (not raw Bass): its compile() splits sync-waits to satisfy the
    # TRN2 1-wait-per-instruction codegen constraint.
    nc = bacc.Bacc(None, target_bir_lowering=False)
    # xhl rows: dc*128+p <-> feature 256*dc + 128*j + p for DoubleRow plane
    # j; cols 0:2048 = hi planes (j*1024 + tok), 2048:4096 = lo planes.
    xhl = nc.declare_dram_parameter("xhl", [DC * P, 4 * TOK], F8, isOutput=False)
    # whl cols: 0:4608 hi (j*2304 + m), 4608:9216 lo. Values are 64*Wqkv.
    whl = nc.declare_dram_parameter("whl", [DC * P, 4 * 3 * D], F8, isOutput=False)
    bqkv = nc.declare_dram_parameter("bqkv", [3 * D], F32, isOutput=False)
    # wohl: DoubleRow pairing of head-pair chunks g: row g*128+p, cols
    # 0:1536 hi (j*768+f), 1536:3072 lo. Values are 64*Wo.
    wohl = nc.declare_dram_parameter("wohl", [DC * P, 4 * D], F8, isOutput=False)
    bo = nc.declare_dram_parameter("bo", [D], F32, isOutput=False)
    out = nc.declare_dram_parameter("out", [TOK, D], F32, isOutput=True)

    with tile.TileContext(nc) as tc:
        _body(tc, xhl, whl, bqkv, wohl, bo, out)
    nc.compile()
    return nc


def _body(tc, xhl, whl, bqkv, wohl, bo, out):
    nc = tc.nc
    AOP = mybir.AluOpType
    ACTF = mybir.ActivationFunctionType

    with (
        tc.tile_pool(name="consts", bufs=1) as consts,
        tc.tile_pool(name="work", bufs=2) as work,
        tc.tile_pool(name="psum", bufs=1, space="PSUM") as psum,
    ):
        # ---- persistent SBUF tensors -------------------------------------
        x_sb = [consts.tile([P, 4 * TOK], F8, tag=f"x{c}", name=f"x{c}") for c in range(DC)]
        w_sb = [consts.tile([P, 4 * 3 * D], F8, tag=f"w{c}", name=f"w{c}") for c in range(DC)]
        wo_sb = [consts.tile([P, 4 * D], F8, tag=f"wo{k}", name=f"wo{k}") for k in range(DC)]
        bqk_sb = consts.tile([P, 2 * D // P], F32, tag="bqk")
        bv_sb = consts.tile([P, D], F32, tag="bv")
        bo_sb = consts.tile([P, D], F32, tag="bo")
        qkT = [consts.tile([P, TOK], BF16, tag=f"qkT{m}", name=f"qkT{m}") for m in range(2 * D // P)]
        # v_aug[t]: per head h, cols 128h..128h+64 = v values, 128h+64.. = 1.0
        vaug = [consts.tile([P, H * 2 * DH], BF16, tag=f"vaug{t}", name=f"vaug{t}") for t in range(TT)]
        ctxT = [consts.tile([P, N], BF16, tag=f"ctxT{i}", name=f"ctxT{i}") for i in range(BPC * HPAIRS)]
        # fp8 hi/lo planes of ctxT for the DoubleRow out-projection; tile g
        # pairs head-pairs (2g, 2g+1) as the two DoubleRow planes.
        ctx8h = [consts.tile([P, 2 * N], F8, tag=f"c8h{i}", name=f"c8h{i}") for i in range(BPC * DC)]
        ctx8l = [consts.tile([P, 2 * N], F8, tag=f"c8l{i}", name=f"c8l{i}") for i in range(BPC * DC)]

        # DoubleRow views: [P, plane_hl(2), plane_dr(2), cols]
        xv = [t.rearrange("p (a j x) -> p a j x", a=2, j=2) for t in x_sb]
        wv = [t.rearrange("p (a j x) -> p a j x", a=2, j=2) for t in w_sb]
        wov = [t.rearrange("p (a j x) -> p a j x", a=2, j=2) for t in wo_sb]
        c8hv = [t.rearrange("p (j x) -> p j x", j=2) for t in ctx8h]
        c8lv = [t.rearrange("p (j x) -> p j x", j=2) for t in ctx8l]
        # (hi,hi), (hi,lo), (lo,hi) term pairs for x@W
        TERMS = ((0, 0), (0, 1), (1, 0))

        # ---- loads. SP ring: x planes (fine first slice so v_proj(0)
        # unblocks early) then the whl q/k columns. ACT ring (idle early):
        # whl v columns. SWDGE (gpsimd) ring: wo + biases.
        # token split at 512: both halves have 512B-contiguous runs (no
        # sub-512B DMA latency penalty); the first half covers v_proj(0..3).
        for c in range(DC):
            nc.sync.dma_start(
                out=xv[c][:, :, :, 0:512],
                in_=xhl[c * P:(c + 1) * P].rearrange(
                    "p (a j x) -> p a j x", a=2, j=2)[:, :, :, 0:512])
        # v columns split across all three DMA rings (transfers overlap
        # cross-ring): chunk 0 on ACT, chunks 1-2 on the SWDGE ring whose
        # transfers run on the separate DMASW track.
        for c, eng in ((0, nc.gpsimd), (2, nc.gpsimd), (1, nc.scalar)):
            eng.dma_start(
                out=wv[c][:, :, :, 2 * D:3 * D],
                in_=whl[c * P:(c + 1) * P].rearrange(
                    "p (a j x) -> p a j x", a=2, j=2)[:, :, :, 2 * D:3 * D])
        # x token upper halves on the ACT ring (free after w0v): keeps the
        # SP queue clear for the q/k weight columns qk_proj(0) needs.
        for c in range(DC):
            nc.scalar.dma_start(
                out=xv[c][:, :, :, 512:TOK],
                in_=xhl[c * P:(c + 1) * P].rearrange(
                    "p (a j x) -> p a j x", a=2, j=2)[:, :, :, 512:TOK])
        for c in range(DC):
            nc.sync.dma_start(
                out=wv[c][:, :, :, 0:2 * D],
                in_=whl[c * P:(c + 1) * P].rearrange(
                    "p (a j x) -> p a j x", a=2, j=2)[:, :, :, 0:2 * D])
        # q/k bias, per-partition layout: bqk_sb[p, m] = bqkv[m*128 + p]
        nc.gpsimd.dma_start(
            out=bqk_sb, in_=bqkv[0:2 * D].rearrange("(m p) -> p m", p=P))
        # v / out biases broadcast along partitions
        bqkv_ap = bqkv[:]
        nc.gpsimd.dma_start(
            out=bv_sb,
            in_=bass.AP(tensor=bqkv_ap.tensor, offset=2 * D, ap=[[0, P], [1, D]]))
        bo_ap = bo[:]
        nc.gpsimd.dma_start(
            out=bo_sb,
            in_=bass.AP(tensor=bo_ap.tensor, offset=0, ap=[[0, P], [1, D]]))
        # Pre-observe the bias DMAs on the engines that consume them, so the
        # hot-loop STT/activation ops carry only their PE wait (walrus's
        # per-instruction sync-wait budget is 1 for STT).
        scratch = consts.tile([1, 4], F32, tag="scratch")
        nc.vector.tensor_copy(out=scratch[0:1, 0:1], in_=bv_sb[0:1, 0:1])
        nc.vector.tensor_copy(out=scratch[0:1, 1:2], in_=bo_sb[0:1, 0:1])
        nc.scalar.copy(out=scratch[0:1, 2:3], in_=bqk_sb[0:1, 0:1])
        # wo on the SWDGE (gpsimd) ring: keeps the SP HWDGE ring free for the
        # x/w loads the first matmuls block on.
        for k in range(DC):
            nc.gpsimd.dma_start(out=wo_sb[k], in_=wohl[k * P:(k + 1) * P, :])

        # ---- phase B0: v-projection (fp8 DoubleRow, 3 terms) -------------
        def v_proj(t):
            # ps1+ps2 packed into one 2-bank "sc" tile (scores are idle in
            # the v phase): a single 768-wide STT drains it, shortening the
            # DVE chain that frees the rotation for tile t+2.
            if t % 2 == 0:
                vps = psum.tile([P, 2 * N], F32, tag="sc", bufs=2)
                ps1 = vps[:, 0:512]
                ps2 = vps[:, 512:768]
            else:
                ps1 = psum.tile([P, 512], F32, tag="mm", bufs=4)
                ps2 = psum.tile([P, 256], F32, tag="mm", bufs=4)
            n9 = len(TERMS) * DC
            i = 0
            for (a, b_) in TERMS:
                for c in range(DC):
                    lhsT = xv[c][:, a, :, t * P:(t + 1) * P]
                    nc.tensor.matmul(ps1, lhsT, wv[c][:, b_, :, 2 * D:2 * D + 512],
                                     start=(i == 0), stop=(i == n9 - 1), perf_mode=DR)
                    nc.tensor.matmul(ps2, lhsT, wv[c][:, b_, :, 2 * D + 512:3 * D],
                                     start=(i == 0), stop=(i == n9 - 1), perf_mode=DR)
                    i += 1
            vview = vaug[t].rearrange("p (h x) -> p h x", x=2 * DH)
            bview = bv_sb.rearrange("p (h x) -> p h x", x=DH)
            if t % 2 == 0:
                nc.vector.scalar_tensor_tensor(
                    out=vview[:, :, 0:DH],
                    in0=vps[:, 0:768].rearrange("p (h x) -> p h x", x=DH),
                    scalar=1.0 / WS, in1=bview,
                    op0=AOP.mult, op1=AOP.add)
            else:
                nc.vector.scalar_tensor_tensor(
                    out=vview[:, 0:8, 0:DH],
                    in0=ps1.rearrange("p (h x) -> p h x", x=DH),
                    scalar=1.0 / WS, in1=bview[:, 0:8, :],
                    op0=AOP.mult, op1=AOP.add)
                nc.vector.scalar_tensor_tensor(
                    out=vview[:, 8:12, 0:DH],
                    in0=ps2.rearrange("p (h x) -> p h x", x=DH),
                    scalar=1.0 / WS, in1=bview[:, 8:12, :],
                    op0=AOP.mult, op1=AOP.add)

        # ---- phase A: q/k projection (fp8 DoubleRow, 3 terms) ------------
        def qk_proj(hp):
            # batch-0 token halves (tch=0) of both q and k first, so the
            # first attention pair unblocks one psum-group earlier.
            for tch in range(2):
                for m in (hp, HPAIRS + hp):
                    ps = psum.tile([P, 512], F32, tag="mm", bufs=4)
                    n9 = len(TERMS) * DC
                    i = 0
                    for (a, b_) in TERMS:
                        for c in range(DC):
                            nc.tensor.matmul(
                                ps,
                                wv[c][:, b_, :, m * P:(m + 1) * P],
                                xv[c][:, a, :, tch * 512:(tch + 1) * 512],
                                start=(i == 0), stop=(i == n9 - 1), perf_mode=DR)
                            i += 1
                    nc.scalar.activation(
                        out=qkT[m][:, tch * 512:(tch + 1) * 512], in_=ps,
                        func=ACTF.Identity, bias=bqk_sb[:, m:m + 1], scale=1.0 / WS)

        # ---- phases C+D per batch (bf16, unchanged math) -----------------
        def scores_kc(b, hp, kc):
            # one 2-bank PSUM tile per kc: hh=0 in cols 0:512, hh=1 in
            # 512:1024, exp'd with a single [128,1024] activation.
            ktile, qtile = qkT[HPAIRS + hp], qkT[hp]
            psb = psum.tile([P, 2 * N], F32, tag="sc", bufs=2)
            for hh in range(2):
                pr = slice(64 * hh, 64 * hh + 64)
                nc.tensor.matmul(
                    psb[:, hh * N:(hh + 1) * N],
                    ktile[pr, b * N + kc * P: b * N + (kc + 1) * P],
                    qtile[pr, b * N:(b + 1) * N],
                    start=True, stop=True)
            at = work.tile([P, 2 * N], BF16, tag="attn", bufs=12)
            nc.scalar.activation(out=at, in_=psb, func=ACTF.Exp,
                                 scale=1.0 / np.sqrt(DH))
            return at

        def scores_emit(b, hp):
            return {kc: scores_kc(b, hp, kc) for kc in range(4)}

        def ctx_half(b, hp, attn, hh):
            h = 2 * hp + hh
            ps_c = psum.tile([P, N], F32, tag="mm", bufs=4)
            for kc in range(4):
                nc.tensor.matmul(
                    ps_c,
                    vaug[b * 4 + kc][:, 2 * DH * h: 2 * DH * (h + 1)],
                    attn[kc][:, hh * N:(hh + 1) * N],
                    start=(kc == 0), stop=(kc == 3))
            bc = work.tile([64, N], F32, tag="bc", bufs=8)
            nc.vector.reciprocal(out=bc, in_=ps_c[64:128, :])
            nc.vector.scalar_tensor_tensor(
                out=ctxT[b * HPAIRS + hp][64 * hh:64 * hh + 64, :],
                in0=ps_c[0:64, :], scalar=1.0, in1=bc,
                op0=AOP.mult, op1=AOP.mult)

        def ctx_tail(b, hp):
            # fp8 hi/lo split for the DoubleRow out-projection, on the idle
            # Pool engine mid-kernel; the final pair sits on the serial tail
            # before out_proj(1,*), so it takes the faster DVE instead.
            g, j = hp // 2, hp % 2
            eng = nc.vector if (b == 1 and hp == HPAIRS - 1) else nc.gpsimd
            hi8 = c8hv[b * DC + g][:, j, :]
            eng.tensor_copy(out=hi8, in_=ctxT[b * HPAIRS + hp])
            eng.tensor_tensor(
                out=c8lv[b * DC + g][:, j, :],
                in0=ctxT[b * HPAIRS + hp], in1=hi8, op=AOP.subtract)

        def ctx_emit(b, hp, attn):
            ctx_half(b, hp, attn, 0)
            ctx_half(b, hp, attn, 1)
            ctx_tail(b, hp)

        def out_proj(b, tt_in_b, fine=False):
            t = b * 4 + tt_in_b
            ps1 = psum.tile([P, 512], F32, tag="mm", bufs=4)
            ps2 = psum.tile([P, 256], F32, tag="mm", bufs=4)
            ts_ = slice(tt_in_b * P, (tt_in_b + 1) * P)
            n9 = len(TERMS) * DC
            if fine:
                # de-interleaved groups: ps1 stops 9 matmuls before PE's
                # end, so its STT+DMA chain overlaps ps2's matmuls and only
                # the small ps2 chunk remains on the end-of-kernel chain.
                for ps, cols in ((ps1, (0, 512)), (ps2, (512, D))):
                    i = 0
                    for g in range(DC):
                        for (a, b_) in TERMS:
                            cv = c8hv if a == 0 else c8lv
                            lhsT = cv[b * DC + g][:, :, ts_]
                            nc.tensor.matmul(
                                ps, lhsT, wov[g][:, b_, :, cols[0]:cols[1]],
                                start=(i == 0), stop=(i == n9 - 1), perf_mode=DR)
                            i += 1
            else:
                i = 0
                for g in range(DC):
                    for (a, b_) in TERMS:
                        cv = c8hv if a == 0 else c8lv
                        lhsT = cv[b * DC + g][:, :, ts_]
                        nc.tensor.matmul(ps1, lhsT, wov[g][:, b_, :, 0:512],
                                         start=(i == 0), stop=(i == n9 - 1), perf_mode=DR)
                        nc.tensor.matmul(ps2, lhsT, wov[g][:, b_, :, 512:D],
                                         start=(i == 0), stop=(i == n9 - 1), perf_mode=DR)
                        i += 1
            # bufs=8: one tile per token tile, so the STT never carries a
            # WAR wait against the previous DMA-out (STT wait budget is 1).
            o = work.tile([P, D], F32, tag="out", bufs=8)
            if not fine:
                chunks = [(ps1, 0, 512, nc.sync), (ps2, 512, D, nc.sync)]
            else:
                # final tile: smaller pieces on alternating DMA rings so the
                # end-of-kernel STT->DMA chain is as short as possible.
                # the ps2 chunk stops last, so its STT->DMA is the tail-
                # critical chain: route it via SWDGE (no HWDGE fixed cost or
                # DGE delay, ~780ns shorter) on the end-phase-idle Pool.
                chunks = [(ps1, 0, 256, nc.sync), (ps1, 256, 512, nc.scalar),
                          (ps2, 512, D, nc.gpsimd)]
            for ci, (ps, lo, hi_, eng) in enumerate(chunks):
                stt_eng = nc.vector
                stt_eng.scalar_tensor_tensor(
                    out=o[:, lo:hi_], in0=ps[:, lo - (0 if ps is ps1 else 512):hi_ - (0 if ps is ps1 else 512)],
                    scalar=1.0 / WS, in1=bo_sb[:, lo:hi_],
                    op0=AOP.mult, op1=AOP.add)
                eng.dma_start(out=out[t * P:(t + 1) * P, lo:hi_], in_=o[:, lo:hi_])

        # Software-pipelined emission: each pair's scores are emitted one
        # stage ahead of its ctx matmuls, so the exp (ScalarE) latency of
        # pair p is hidden behind the qk_proj / out_proj / ctx PE work
        # emitted in between.
        for t in range(TT):
            v_proj(t)
        # ones columns of v_aug (persistent; written once): emitted AFTER the
        # v projections so the static DVE order runs the rotation-freeing v
        # STTs first; the memsets fill DVE idle before attention needs them.
        for t in range(TT):
            ones_view = vaug[t].rearrange("p (h x) -> p h x", x=2 * DH)[:, :, DH:2 * DH]
            nc.vector.memset(ones_view, 1.0)
        attn_q = []  # queue of (b, hp, attn) awaiting ctx
        qk_proj(0)
        attn_q.append((0, 0, scores_emit(0, 0)))
        for hp in range(1, HPAIRS):
            qk_proj(hp)
            b0, hp0, at0 = attn_q.pop(0)
            ctx_emit(b0, hp0, at0)
            attn_q.append((0, hp, scores_emit(0, hp)))
        # phase B: kc-granular interleave — the sc-PSUM WAR wait of pair
        # p+1's later kc tiles is absorbed by the ctx matmuls of pair p
        # instead of head-of-line blocking the PE queue.
        for hp in range(HPAIRS):
            b0, hp0, at0 = attn_q[0]
            at = {}
            at[0] = scores_kc(1, hp, 0)
            at[1] = scores_kc(1, hp, 1)
            ctx_half(b0, hp0, at0, 0)
            at[2] = scores_kc(1, hp, 2)
            at[3] = scores_kc(1, hp, 3)
            ctx_half(b0, hp0, at0, 1)
            ctx_tail(b0, hp0)
            attn_q.pop(0)
            attn_q.append((1, hp, at))
            if hp >= 2:
                out_proj(0, hp - 2)
        b0, hp0, at0 = attn_q.pop(0)
        ctx_emit(b0, hp0, at0)
        for tt_in_b in range(4):
            out_proj(1, tt_in_b, fine=True)



def _get_nc():
    global _CACHED_NC
    if _CACHED_NC is None:
        _CACHED_NC = _build_nc()
    return _CACHED_NC


def _split_hl(a):
    hi = a.astype(F8_NP)
    lo = (a - hi.astype(np.float32)).astype(F8_NP)
    return hi, lo


def _dr_layout(a):
    """[D, C] fp8 -> [DC*P, 2*C]: row dc*128+p, col j*C+c holds
    a[256*dc + 128*j + p, c] (DoubleRow plane pairing along features)."""
    dfull, cols = a.shape
    return np.ascontiguousarray(
        a.reshape(DC, 2, P, cols).transpose(0, 2, 1, 3).reshape(DC * P, 2 * cols))


def make_in_maps(inputs):
    x = np.asarray(inputs["x"], dtype=np.float32)
    wqkv = np.asarray(inputs["Wqkv"], dtype=np.float32)
    wo_f = np.asarray(inputs["Wo"], dtype=np.float32)
    bqkv_f = np.ascontiguousarray(np.asarray(inputs["bqkv"], dtype=np.float32))
    bo_f = np.ascontiguousarray(np.asarray(inputs["bo"], dtype=np.float32))

    wh, wl = _split_hl(wqkv * WS)
    whl = np.concatenate([_dr_layout(wh), _dr_layout(wl)], axis=1)
    woh, wol = _split_hl(wo_f * WS)
    wohl = np.concatenate([_dr_layout(woh), _dr_layout(wol)], axis=1)

    in_maps = []
    for c in range(NCORES):
        xc = np.ascontiguousarray(
            x[c * BPC:(c + 1) * BPC].reshape(TOK, D).T)  # [768, 1024]
        xh, xl = _split_hl(xc)
        xhl = np.concatenate([_dr_layout(xh), _dr_layout(xl)], axis=1)
        in_maps.append({
            "xhl": xhl,
            "whl": whl,
            "bqkv": bqkv_f,
            "wohl": wohl,
            "bo": bo_f,
        })
    return in_maps


def kernel(x, Wqkv, bqkv, Wo, bo):
    global LAST_EXEC_NS, LAST_RESULTS
    in_maps = make_in_maps(
        {"x": x, "Wqkv": Wqkv, "bqkv": bqkv, "Wo": Wo, "bo": bo})

    nc = _get_nc()
    res = run_bass_kernel_spmd(nc, in_maps, list(range(NCORES)), trace=TRACE)
    LAST_EXEC_NS = res.exec_time_ns
    LAST_RESULTS = res
    outs = [np.asarray(res.results[c]["out"], dtype=np.float32) for c in range(NCORES)]
    return np.concatenate(outs, axis=0).reshape(B, N, D)


# revision 67
# speedup vs baseline: 1.0015x; 1.0015x over previous
"""Multi-head attention (b=16, n=512, d=768, h=12) on 8 trn2 NeuronCores.

Strategy: pure data-parallel over batch (2 batches per core), no collectives.

QKV projection runs in fp8e4m3 DoubleRow mode (2x contraction per partition,
0.5 PE cycles per output row): host splits x and 64*Wqkv into hi+lo e4m3
planes; the projection computes xh@wh + xh@wl + xl@wh (the dropped lo@lo
term is ~0.06% relative), a 1.33x PE saving over bf16 at bf16-level accuracy.
The 64x weight pre-scale keeps the lo planes out of e4m3's subnormal range;
the 1/64 unscale rides the existing PSUM->SBUF copies for free.

Per-core dataflow (P = 128 partitions):
  qkT[m]  = (Wqkv hi/lo)^T @ (x hi/lo)  DoubleRow -> [feat, tok] bf16 (+bias)
  v_aug   = x @ Wv stored per head as [v_h | ones64]  (natural [tok, feat])
  scoresT = k_h @ q_h^T   (bf16, 2 heads -> one 2-bank PSUM tile)
  attnT   = exp(0.125 * scoresT)  (one [128,1024] activation per 2 banks)
  ctx_h   = v_aug_h^T @ attnT: rows 0-63 = ctxT, rows 64-127 = colsum
  bc      = 1/colsum; ctxT = ctx * bc (fused on the PSUM->SBUF copy)
  out     = ctxT^T @ Wo + bo  (bf16, natural [tok, feat], DMA out)
"""

import numpy as np
import ml_dtypes

import concourse.bass as bass
import concourse.mybir as mybir
import concourse.tile as tile
from concourse import bacc
from concourse.bass_utils import run_bass_kernel_spmd

# Problem constants (hardcoded per contest contract).
B = 16          # global batch
N = 512         # sequence length
D = 768         # embed dim
H = 12          # heads
DH = 64         # head dim
NCORES = 8
BPC = B // NCORES          # batches per core = 2
TOK = BPC * N              # tokens per core = 1024
P = 128
DC = 3                     # fp8 DoubleRow double-chunks over D (3 x 256)
TT = TOK // P              # 8 token tiles
HPAIRS = H // 2            # 6 head pairs
WS = 64.0                  # weight pre-scale (keeps fp8 lo plane normal)

F32 = mybir.dt.float32
BF16 = mybir.dt.bfloat16
F8 = mybir.dt.float8e4
BF16_NP = ml_dtypes.bfloat16
F8_NP = ml_dtypes.float8_e4m3
DR = mybir.MatmulPerfMode.DoubleRow

# Module-level knobs (test.py pokes these; harness uses defaults).
TRACE = False
LAST_EXEC_NS = None
LAST_RESULTS = None

_CACHED_NC = None


def _build_nc():
    # Bacc (not raw Bass): its compile() splits sync-waits to satisfy the
    # TRN2 1-wait-per-instruction codegen constraint.
    nc = bacc.Bacc(None, target_bir_lowering=False)
    # xhl rows: dc*128+p <-> feature 256*dc + 128*j + p for DoubleRow plane
    # j; cols 0:2048 = hi planes (j*1024 + tok), 2048:4096 = lo planes.
    xhl = nc.declare_dram_parameter("xhl", [DC * P, 4 * TOK], F8, isOutput=False)
    # whl cols: 0:4608 hi (j*2304 + m), 4608:9216 lo. Values are 64*Wqkv.
    whl = nc.declare_dram_parameter("whl", [DC * P, 4 * 3 * D], F8, isOutput=False)
    bqkv = nc.declare_dram_parameter("bqkv", [3 * D], F32, isOutput=False)
    # wohl: DoubleRow pairing of head-pair chunks g: row g*128+p, cols
    # 0:1536 hi (j*768+f), 1536:3072 lo. Values are 64*Wo.
    wohl = nc.declare_dram_parameter("wohl", [DC * P, 4 * D], F8, isOutput=False)
    bo = nc.declare_dram_parameter("bo", [D], F32, isOutput=False)
    out = nc.declare_dram_parameter("out", [TOK, D], F32, isOutput=True)

    with tile.TileContext(nc) as tc:
        _body(tc, xhl, whl, bqkv, wohl, bo, out)
    nc.compile()
    return nc


def _body(tc, xhl, whl, bqkv, wohl, bo, out):
    nc = tc.nc
    AOP = mybir.AluOpType
    ACTF = mybir.ActivationFunctionType

    with (
        tc.tile_pool(name="consts", bufs=1) as consts,
        tc.tile_pool(name="work", bufs=2) as work,
        tc.tile_pool(name="psum", bufs=1, space="PSUM") as psum,
    ):
        # ---- persistent SBUF tensors -------------------------------------
        x_sb = [consts.tile([P, 4 * TOK], F8, tag=f"x{c}", name=f"x{c}") for c in range(DC)]
        w_sb = [consts.tile([P, 4 * 3 * D], F8, tag=f"w{c}", name=f"w{c}") for c in range(DC)]
        wo_sb = [consts.tile([P, 4 * D], F8, tag=f"wo{k}", name=f"wo{k}") for k in range(DC)]
        bqk_sb = consts.tile([P, 2 * D // P], F32, tag="bqk")
        bv_sb = consts.tile([P, D], F32, tag="bv")
        bo_sb = consts.tile([P, D], F32, tag="bo")
        qkT = [consts.tile([P, TOK], BF16, tag=f"qkT{m}", name=f"qkT{m}") for m in range(2 * D // P)]
        # v_aug[t]: per head h, cols 128h..128h+64 = v values, 128h+64.. = 1.0
        vaug = [consts.tile([P, H * 2 * DH], BF16, tag=f"vaug{t}", name=f"vaug{t}") for t in range(TT)]
        ctxT = [consts.tile([P, N], BF16, tag=f"ctxT{i}", name=f"ctxT{i}") for i in range(BPC * HPAIRS)]
        # fp8 hi/lo planes of ctxT for the DoubleRow out-projection; tile g
        # pairs head-pairs (2g, 2g+1) as the two DoubleRow planes.
        ctx8h = [consts.tile([P, 2 * N], F8, tag=f"c8h{i}", name=f"c8h{i}") for i in range(BPC * DC)]
        ctx8l = [consts.tile([P, 2 * N], F8, tag=f"c8l{i}", name=f"c8l{i}") for i in range(BPC * DC)]

        # DoubleRow views: [P, plane_hl(2), plane_dr(2), cols]
        xv = [t.rearrange("p (a j x) -> p a j x", a=2, j=2) for t in x_sb]
        wv = [t.rearrange("p (a j x) -> p a j x", a=2, j=2) for t in w_sb]
        wov = [t.rearrange("p (a j x) -> p a j x", a=2, j=2) for t in wo_sb]
        c8hv = [t.rearrange("p (j x) -> p j x", j=2) for t in ctx8h]
        c8lv = [t.rearrange("p (j x) -> p j x", j=2) for t in ctx8l]
        # (hi,hi), (hi,lo), (lo,hi) term pairs for x@W
        TERMS = ((0, 0), (0, 1), (1, 0))

        # ---- loads. SP ring: x planes (fine first slice so v_proj(0)
        # unblocks early) then the whl q/k columns. ACT ring (idle early):
        # whl v columns. SWDGE (gpsimd) ring: wo + biases.
        # token split at 512: both halves have 512B-contiguous runs (no
        # sub-512B DMA latency penalty); the first half covers v_proj(0..3).
        for c in range(DC):
            nc.sync.dma_start(
                out=xv[c][:, :, :, 0:512],
                in_=xhl[c * P:(c + 1) * P].rearrange(
                    "p (a j x) -> p a j x", a=2, j=2)[:, :, :, 0:512])
        # v columns split across all three DMA rings (transfers overlap
        # cross-ring): chunk 0 on ACT, chunks 1-2 on the SWDGE ring whose
        # transfers run on the separate DMASW track.
        for c, eng in ((0, nc.gpsimd), (2, nc.gpsimd), (1, nc.scalar)):
            eng.dma_start(
                out=wv[c][:, :, :, 2 * D:3 * D],
                in_=whl[c * P:(c + 1) * P].rearrange(
                    "p (a j x) -> p a j x", a=2, j=2)[:, :, :, 2 * D:3 * D])
        # x token upper halves on the ACT ring (free after w0v): keeps the
        # SP queue clear for the q/k weight columns qk_proj(0) needs.
        for c in range(DC):
            nc.scalar.dma_start(
                out=xv[c][:, :, :, 512:TOK],
                in_=xhl[c * P:(c + 1) * P].rearrange(
                    "p (a j x) -> p a j x", a=2, j=2)[:, :, :, 512:TOK])
        for c in range(DC):
            nc.sync.dma_start(
                out=wv[c][:, :, :, 0:2 * D],
                in_=whl[c * P:(c + 1) * P].rearrange(
                    "p (a j x) -> p a j x", a=2, j=2)[:, :, :, 0:2 * D])
        # q/k bias, per-partition layout: bqk_sb[p, m] = bqkv[m*128 + p]
        nc.gpsimd.dma_start(
            out=bqk_sb, in_=bqkv[0:2 * D].rearrange("(m p) -> p m", p=P))
        # v / out biases broadcast along partitions
        bqkv_ap = bqkv[:]
        nc.gpsimd.dma_start(
            out=bv_sb,
            in_=bass.AP(tensor=bqkv_ap.tensor, offset=2 * D, ap=[[0, P], [1, D]]))
        bo_ap = bo[:]
        nc.gpsimd.dma_start(
            out=bo_sb,
            in_=bass.AP(tensor=bo_ap.tensor, offset=0, ap=[[0, P], [1, D]]))
        # Pre-observe the bias DMAs on the engines that consume them, so the
        # hot-loop STT/activation ops carry only their PE wait (walrus's
        # per-instruction sync-wait budget is 1 for STT).
        scratch = consts.tile([1, 4], F32, tag="scratch")
        nc.vector.tensor_copy(out=scratch[0:1, 0:1], in_=bv_sb[0:1, 0:1])
        nc.vector.tensor_copy(out=scratch[0:1, 1:2], in_=bo_sb[0:1, 0:1])
        nc.scalar.copy(out=scratch[0:1, 2:3], in_=bqk_sb[0:1, 0:1])
        # wo on the SWDGE (gpsimd) ring: keeps the SP HWDGE ring free for the
        # x/w loads the first matmuls block on.
        for k in range(DC):
            nc.gpsimd.dma_start(out=wo_sb[k], in_=wohl[k * P:(k + 1) * P, :])

        # ---- phase B0: v-projection (fp8 DoubleRow, 3 terms) -------------
        def v_proj(t):
            # ps1+ps2 packed into one 2-bank "sc" tile (scores are idle in
            # the v phase): a single 768-wide STT drains it, shortening the
            # DVE chain that frees the rotation for tile t+2.
            if t % 2 == 0:
                vps = psum.tile([P, 2 * N], F32, tag="sc", bufs=2)
                ps1 = vps[:, 0:512]
                ps2 = vps[:, 512:768]
            else:
                ps1 = psum.tile([P, 512], F32, tag="mm", bufs=4)
                ps2 = psum.tile([P, 256], F32, tag="mm", bufs=4)
            n9 = len(TERMS) * DC
            i = 0
            for (a, b_) in TERMS:
                for c in range(DC):
                    lhsT = xv[c][:, a, :, t * P:(t + 1) * P]
                    nc.tensor.matmul(ps1, lhsT, wv[c][:, b_, :, 2 * D:2 * D + 512],
                                     start=(i == 0), stop=(i == n9 - 1), perf_mode=DR)
                    nc.tensor.matmul(ps2, lhsT, wv[c][:, b_, :, 2 * D + 512:3 * D],
                                     start=(i == 0), stop=(i == n9 - 1), perf_mode=DR)
                    i += 1
            vview = vaug[t].rearrange("p (h x) -> p h x", x=2 * DH)
            bview = bv_sb.rearrange("p (h x) -> p h x", x=DH)
            if t % 2 == 0:
                nc.vector.scalar_tensor_tensor(
                    out=vview[:, :, 0:DH],
                    in0=vps[:, 0:768].rearrange("p (h x) -> p h x", x=DH),
                    scalar=1.0 / WS, in1=bview,
                    op0=AOP.mult, op1=AOP.add)
            else:
                nc.vector.scalar_tensor_tensor(
                    out=vview[:, 0:8, 0:DH],
                    in0=ps1.rearrange("p (h x) -> p h x", x=DH),
                    scalar=1.0 / WS, in1=bview[:, 0:8, :],
                    op0=AOP.mult, op1=AOP.add)
                nc.vector.scalar_tensor_tensor(
                    out=vview[:, 8:12, 0:DH],
                    in0=ps2.rearrange("p (h x) -> p h x", x=DH),
                    scalar=1.0 / WS, in1=bview[:, 8:12, :],
                    op0=AOP.mult, op1=AOP.add)

        # ---- phase A: q/k projection (fp8 DoubleRow, 3 terms) ------------
        def qk_proj(hp):
            # batch-0 token halves (tch=0) of both q and k first, so the
            # first attention pair unblocks one psum-group earlier.
            for tch in range(2):
                for m in (hp, HPAIRS + hp):
                    ps = psum.tile([P, 512], F32, tag="mm", bufs=4)
                    n9 = len(TERMS) * DC
                    i = 0
                    for (a, b_) in TERMS:
                        for c in range(DC):
                            nc.tensor.matmul(
                                ps,
                                wv[c][:, b_, :, m * P:(m + 1) * P],
                                xv[c][:, a, :, tch * 512:(tch + 1) * 512],
                                start=(i == 0), stop=(i == n9 - 1), perf_mode=DR)
                            i += 1
                    nc.scalar.activation(
                        out=qkT[m][:, tch * 512:(tch + 1) * 512], in_=ps,
                        func=ACTF.Identity, bias=bqk_sb[:, m:m + 1], scale=1.0 / WS)

        # ---- phases C+D per batch (bf16, unchanged math) -----------------
        def scores_kc(b, hp, kc, fine_exp=False):
            # one 2-bank PSUM tile per kc: hh=0 in cols 0:512, hh=1 in
            # 512:1024, exp'd with a single [128,1024] activation (or two
            # halves when fine_exp: shorter serial latency for the final
            # pairs, where no other PE work can cover the exp wait).
            ktile, qtile = qkT[HPAIRS + hp], qkT[hp]
            psb = psum.tile([P, 2 * N], F32, tag="sc", bufs=2)
            for hh in range(2):
                pr = slice(64 * hh, 64 * hh + 64)
                nc.tensor.matmul(
                    psb[:, hh * N:(hh + 1) * N],
                    ktile[pr, b * N + kc * P: b * N + (kc + 1) * P],
                    qtile[pr, b * N:(b + 1) * N],
                    start=True, stop=True)
            at = work.tile([P, 2 * N], BF16, tag="attn", bufs=12)
            if fine_exp:
                for hh in range(2):
                    nc.scalar.activation(
                        out=at[:, hh * N:(hh + 1) * N],
                        in_=psb[:, hh * N:(hh + 1) * N], func=ACTF.Exp,
                        scale=1.0 / np.sqrt(DH))
            else:
                nc.scalar.activation(out=at, in_=psb, func=ACTF.Exp,
                                     scale=1.0 / np.sqrt(DH))
            return at

        def scores_emit(b, hp):
            return {kc: scores_kc(b, hp, kc) for kc in range(4)}

        def ctx_half(b, hp, attn, hh):
            h = 2 * hp + hh
            ps_c = psum.tile([P, N], F32, tag="mm", bufs=4)
            for kc in range(4):
                nc.tensor.matmul(
                    ps_c,
                    vaug[b * 4 + kc][:, 2 * DH * h: 2 * DH * (h + 1)],
                    attn[kc][:, hh * N:(hh + 1) * N],
                    start=(kc == 0), stop=(kc == 3))
            bc = work.tile([64, N], F32, tag="bc", bufs=8)
            nc.vector.reciprocal(out=bc, in_=ps_c[64:128, :])
            nc.vector.scalar_tensor_tensor(
                out=ctxT[b * HPAIRS + hp][64 * hh:64 * hh + 64, :],
                in0=ps_c[0:64, :], scalar=1.0, in1=bc,
                op0=AOP.mult, op1=AOP.mult)

        def ctx_tail(b, hp):
            # fp8 hi/lo split for the DoubleRow out-projection, on the idle
            # Pool engine mid-kernel; the final pair sits on the serial tail
            # before out_proj(1,*), so it takes the faster DVE instead.
            g, j = hp // 2, hp % 2
            eng = nc.vector if (b == 1 and hp == HPAIRS - 1) else nc.gpsimd
            hi8 = c8hv[b * DC + g][:, j, :]
            eng.tensor_copy(out=hi8, in_=ctxT[b * HPAIRS + hp])
            eng.tensor_tensor(
                out=c8lv[b * DC + g][:, j, :],
                in0=ctxT[b * HPAIRS + hp], in1=hi8, op=AOP.subtract)

        def ctx_emit(b, hp, attn):
            ctx_half(b, hp, attn, 0)
            ctx_half(b, hp, attn, 1)
            ctx_tail(b, hp)

        def out_proj(b, tt_in_b, fine=False):
            t = b * 4 + tt_in_b
            ps1 = psum.tile([P, 512], F32, tag="mm", bufs=4)
            ps2 = psum.tile([P, 256], F32, tag="mm", bufs=4)
            ts_ = slice(tt_in_b * P, (tt_in_b + 1) * P)
            n9 = len(TERMS) * DC
            if fine:
                # de-interleaved groups: ps1 stops 9 matmuls before PE's
                # end, so its STT+DMA chain overlaps ps2's matmuls and only
                # the small ps2 chunk remains on the end-of-kernel chain.
                for ps, cols in ((ps1, (0, 512)), (ps2, (512, D))):
                    i = 0
                    for g in range(DC):
                        for (a, b_) in TERMS:
                            cv = c8hv if a == 0 else c8lv
                            lhsT = cv[b * DC + g][:, :, ts_]
                            nc.tensor.matmul(
                                ps, lhsT, wov[g][:, b_, :, cols[0]:cols[1]],
                                start=(i == 0), stop=(i == n9 - 1), perf_mode=DR)
                            i += 1
            else:
                i = 0
                for g in range(DC):
                    for (a, b_) in TERMS:
                        cv = c8hv if a == 0 else c8lv
                        lhsT = cv[b * DC + g][:, :, ts_]
                        nc.tensor.matmul(ps1, lhsT, wov[g][:, b_, :, 0:512],
                                         start=(i == 0), stop=(i == n9 - 1), perf_mode=DR)
                        nc.tensor.matmul(ps2, lhsT, wov[g][:, b_, :, 512:D],
                                         start=(i == 0), stop=(i == n9 - 1), perf_mode=DR)
                        i += 1
            # bufs=8: one tile per token tile, so the STT never carries a
            # WAR wait against the previous DMA-out (STT wait budget is 1).
            o = work.tile([P, D], F32, tag="out", bufs=8)
            if not fine:
                chunks = [(ps1, 0, 512, nc.sync), (ps2, 512, D, nc.sync)]
            else:
                # final tile: smaller pieces on alternating DMA rings so the
                # end-of-kernel STT->DMA chain is as short as possible.
                # the ps2 chunk stops last, so its STT->DMA is the tail-
                # critical chain: route it via SWDGE (no HWDGE fixed cost or
                # DGE delay, ~780ns shorter) on the end-phase-idle Pool.
                # both ps1 chunks on the idle SP ring: a scalar-ring DMA
                # issue costs 667ns of Act SEQ time and delays the final
                # pairs' exps (Act is the attention pacer).
                chunks = [(ps1, 0, 256, nc.sync), (ps1, 256, 512, nc.sync),
                          (ps2, 512, D, nc.gpsimd)]
            for ci, (ps, lo, hi_, eng) in enumerate(chunks):
                stt_eng = nc.vector
                stt_eng.scalar_tensor_tensor(
                    out=o[:, lo:hi_], in0=ps[:, lo - (0 if ps is ps1 else 512):hi_ - (0 if ps is ps1 else 512)],
                    scalar=1.0 / WS, in1=bo_sb[:, lo:hi_],
                    op0=AOP.mult, op1=AOP.add)
                eng.dma_start(out=out[t * P:(t + 1) * P, lo:hi_], in_=o[:, lo:hi_])

        # Software-pipelined emission: each pair's scores are emitted one
        # stage ahead of its ctx matmuls, so the exp (ScalarE) latency of
        # pair p is hidden behind the qk_proj / out_proj / ctx PE work
        # emitted in between.
        for t in range(TT):
            v_proj(t)
        # ones columns of v_aug (persistent; written once): emitted AFTER the
        # v projections so the static DVE order runs the rotation-freeing v
        # STTs first; the memsets fill DVE idle before attention needs them.
        for t in range(TT):
            ones_view = vaug[t].rearrange("p (h x) -> p h x", x=2 * DH)[:, :, DH:2 * DH]
            nc.vector.memset(ones_view, 1.0)
        attn_q = []  # queue of (b, hp, attn) awaiting ctx
        qk_proj(0)
        attn_q.append((0, 0, scores_emit(0, 0)))
        for hp in range(1, HPAIRS):
            qk_proj(hp)
            b0, hp0, at0 = attn_q.pop(0)
            ctx_emit(b0, hp0, at0)
            attn_q.append((0, hp, scores_emit(0, hp)))
        # phase B: kc-granular interleave — the sc-PSUM WAR wait of pair
        # p+1's later kc tiles is absorbed by the ctx matmuls of pair p
        # instead of head-of-line blocking the PE queue.
        for hp in range(HPAIRS):
            b0, hp0, at0 = attn_q[0]
            fe = hp >= HPAIRS - 2
            at = {}
            at[0] = scores_kc(1, hp, 0, fe)
            at[1] = scores_kc(1, hp, 1, fe)
            ctx_half(b0, hp0, at0, 0)
            at[2] = scores_kc(1, hp, 2, fe)
            at[3] = scores_kc(1, hp, 3, fe)
            ctx_half(b0, hp0, at0, 1)
            ctx_tail(b0, hp0)
            attn_q.pop(0)
            attn_q.append((1, hp, at))
            if hp >= 2:
                out_proj(0, hp - 2)
        b0, hp0, at0 = attn_q.pop(0)
        ctx_emit(b0, hp0, at0)
        for tt_in_b in range(4):
            out_proj(1, tt_in_b, fine=True)



def _get_nc():
    global _CACHED_NC
    if _CACHED_NC is None:
        _CACHED_NC = _build_nc()
    return _CACHED_NC


def _split_hl(a):
    hi = a.astype(F8_NP)
    lo = (a - hi.astype(np.float32)).astype(F8_NP)
    return hi, lo


def _dr_layout(a):
    """[D, C] fp8 -> [DC*P, 2*C]: row dc*128+p, col j*C+c holds
    a[256*dc + 128*j + p, c] (DoubleRow plane pairing along features)."""
    dfull, cols = a.shape
    return np.ascontiguousarray(
        a.reshape(DC, 2, P, cols).transpose(0, 2, 1, 3).reshape(DC * P, 2 * cols))


def make_in_maps(inputs):
    x = np.asarray(inputs["x"], dtype=np.float32)
    wqkv = np.asarray(inputs["Wqkv"], dtype=np.float32)
    wo_f = np.asarray(inputs["Wo"], dtype=np.float32)
    bqkv_f = np.ascontiguousarray(np.asarray(inputs["bqkv"], dtype=np.float32))
    bo_f = np.ascontiguousarray(np.asarray(inputs["bo"], dtype=np.float32))

    wh, wl = _split_hl(wqkv * WS)
    whl = np.concatenate([_dr_layout(wh), _dr_layout(wl)], axis=1)
    woh, wol = _split_hl(wo_f * WS)
    wohl = np.concatenate([_dr_layout(woh), _dr_layout(wol)], axis=1)

    in_maps = []
    for c in range(NCORES):
        xc = np.ascontiguousarray(
            x[c * BPC:(c + 1) * BPC].reshape(TOK, D).T)  # [768, 1024]
        xh, xl = _split_hl(xc)
        xhl = np.concatenate([_dr_layout(xh), _dr_layout(xl)], axis=1)
        in_maps.append({
            "xhl": xhl,
            "whl": whl,
            "bqkv": bqkv_f,
            "wohl": wohl,
            "bo": bo_f,
        })
    return in_maps


def kernel(x, Wqkv, bqkv, Wo, bo):
    global LAST_EXEC_NS, LAST_RESULTS
    in_maps = make_in_maps(
        {"x": x, "Wqkv": Wqkv, "bqkv": bqkv, "Wo": Wo, "bo": bo})

    nc = _get_nc()
    res = run_bass_kernel_spmd(nc, in_maps, list(range(NCORES)), trace=TRACE)
    LAST_EXEC_NS = res.exec_time_ns
    LAST_RESULTS = res
    outs = [np.asarray(res.results[c]["out"], dtype=np.float32) for c in range(NCORES)]
    return np.concatenate(outs, axis=0).reshape(B, N, D)
